# revision 26
# baseline (speedup 1.0000x reference)
"""TextCNN-style conv layer (kernel sizes 3/4/5, EMB=300 -> DEPTH=256, bias,
ReLU, max-pool over time) as a Bass/Tile kernel for 8 Trainium2 NeuronCores.

Strategy: data-parallel over batch (8 samples per core), weights replicated.

Conv as dense-K matmuls: for branch n, window output
y[d, i] = sum_{k < n*300} Xrep[k, i] * Wn[d, k]  with  Xrep[k, i] =
x[i + k//300, k%300] -- the im2col matrix.  Xrep rows are materialized in
SBUF as 12 K-tiles of 128 rows per sample, built by <=2 shifted DMA segments
per tile straight from the transposed input in DRAM (a row (j, e) is just
x_t[e, j:] -- a free-dim offset), so no host-side replication and each
branch contracts over ceil(n*300/128) dense K=128 tiles (8/10/12 -> 30
matmuls per sample per depth-half vs 36 for the per-(j,chunk) split).
Branch boundaries that fall inside a tile are handled by zero-padding the
*weights* (the x rows there hold valid shifted data).  The final K-tile's
rows past 1500 are never written, so its matmuls contract only K=92.

dtype float32r: FP22 multiplies at full PE rate, fp32 PSUM accumulate; the
moving free-dim count must be even, so branches with odd SEQ-n compute one
extra (still valid) window that the max-reduce then ignores.

Epilogue: relu(max_i(y + b)) == max(0, max_i y + b): DVE reduce_max over the
window axis straight out of PSUM, broadcast bias add + clamp at 0, output
staged [d, branch, half, sample] per core and de-transposed on host.
"""

import numpy as np

B, SEQ, EMB = 64, 394, 300
DEPTH = 256
NCORES = 8
BPC = B // NCORES  # samples per core
SEQP = 400  # x_t free-dim padded (zeros) so shifted loads stay in bounds
NS = (3, 4, 5)
NTILES = (8, 10, 12)  # ceil(n*300/128) K-tiles per branch
COLB = (0, 8, 18)  # weight column base per branch
NCOL = 30
KTOT = 12  # distinct Xrep K-tiles per sample

# DMA segments building the 12 Xrep K-tiles: (tile r, p0, plen, j, e0)
_SEGS = []
for _r in range(KTOT):
    _k, _k1 = 128 * _r, min(128 * (_r + 1), 5 * EMB)
    while _k < _k1:
        _j, _e = divmod(_k, EMB)
        _plen = min(_k1 - _k, EMB - _e)
        _SEGS.append((_r, _k - 128 * _r, _plen, _j, _e))
        _k += _plen

TRACE = False
LAST_RESULT = None

_built = None


def _build_bass():
    import concourse.mybir as mybir
    import concourse.tile as tile
    from concourse import bacc
    from contextlib import ExitStack

    f32 = mybir.dt.float32
    f32r = mybir.dt.float32r
    f16 = mybir.dt.float16

    nc = bacc.Bacc("TRN2", target_bir_lowering=False)
    xt_d = nc.dram_tensor("xt", (BPC, KTOT, 128, SEQP), f16, kind="ExternalInput")
    wq_d = nc.dram_tensor("wq", (128, 2, NCOL, 128), f16, kind="ExternalInput")
    bp_d = nc.dram_tensor("bp", (128, 3, 2), f32, kind="ExternalInput")
    out_d = nc.dram_tensor("out_t", (128, 3, 2, BPC), f32, kind="ExternalOutput")

    with tile.TileContext(nc) as tc, ExitStack() as ctx:
        xpool = ctx.enter_context(tc.tile_pool(name="x", bufs=5))
        wpool = ctx.enter_context(tc.tile_pool(name="w", bufs=1))
        cpool = ctx.enter_context(tc.tile_pool(name="consts", bufs=1))
        spool = ctx.enter_context(tc.tile_pool(name="stage", bufs=1))
        pspool = ctx.enter_context(tc.tile_pool(name="ps", bufs=7, space="PSUM"))
        warmps = ctx.enter_context(tc.tile_pool(name="warmps", bufs=1, space="PSUM"))

        # x segments alternate over the two fast HWDGE rings (SP, ACT);
        # weights + bias go on the gpsimd SWDGE ring in parallel so they
        # never block the x pipeline.
        hw_engines = (nc.sync, nc.scalar)
        rr = [0]

        wts = {}

        def load_w(dh, br, eng):
            nt = NTILES[br]
            wt = wpool.tile([128, nt, 128], f16, tag=f"w{dh}{br}")
            eng.dma_start(wt[:], wq_d[:, dh, COLB[br] : COLB[br] + nt, :])
            wts[dh, br] = wt

        def load_x(s):
            # One pool tile + one contiguous DMA per K-tile: a matmul waits
            # only on the single DMA that wrote its contraction rows.
            xr = [
                xpool.tile([128, SEQP], f16, tag=f"x{r}", name=f"x{r}_{s}")
                for r in range(KTOT)
            ]
            for r in range(KTOT):
                eng = hw_engines[rr[0] % 2]
                rr[0] += 1
                eng.dma_start(xr[r][:], xt_d[s, r])
            return xr

        # The whole working set (3.9MB weights + 2.4MB im2col per sample)
        # drains from HBM at ~350GB/s, so the first ~25us are DMA-paced.
        # Interleave the loads in need-order and run samples 0-1 group-major
        # (stretching each weight tile's deadline) before switching to
        # sample-major for the pipelined steady state.
        # PE pre-warm: dummy matmuls on a zeroed tile run during the initial
        # DMA window so the HAM clock gate is at 2.4GHz (and the PE pipeline
        # hot) when the real matmuls start.
        warm = cpool.tile([128, 640], f16)
        nc.gpsimd.memset(warm[:], 0.0)
        wps = warmps.tile([128, 512], f32)
        for _ in range(8):
            nc.tensor.matmul(
                wps[:], lhsT=warm[:, :128], rhs=warm[:, 128:640],
                start=True, stop=True,
            )

        load_w(0, 0, nc.sync)
        xrs = [load_x(0)]
        load_w(0, 1, nc.scalar)
        xrs.append(load_x(1))
        load_w(0, 2, nc.sync)
        load_w(1, 0, nc.scalar)
        load_w(1, 1, nc.sync)
        load_w(1, 2, nc.scalar)
        xrs.append(load_x(2))
        bt = cpool.tile([128, 3, 2], f32)
        nc.gpsimd.dma_start(bt[:], bp_d[:])
        xrs.append(load_x(3))

        stage = spool.tile([128, 3, 2, BPC], f32)

        def do_group(s, dh, br):
            n = NS[br]
            nw = SEQ - n  # windows the reference maxes over
            nmm = nw + (nw & 1)  # keep the moving count even
            nt = NTILES[br]
            xr = xrs[s]
            ps = pspool.tile([128, 512], f32, tag="ps", name=f"ps_{s}_{dh}_{br}")
            for r in range(nt):
                kk = min(128, 5 * EMB - 128 * r)  # 92 on the last tile
                nc.tensor.matmul(
                    ps[:, :nmm],
                    lhsT=wts[dh, br][:kk, r, :],
                    rhs=xr[r][:kk, :nmm],
                    start=(r == 0),
                    stop=(r == nt - 1),
                )
            nc.vector.reduce_max(
                stage[:, br, dh, s : s + 1],
                ps[:, :nw],
                axis=mybir.AxisListType.X,
            )

        for dh in range(2):
            for br in range(3):
                do_group(0, dh, br)
                do_group(1, dh, br)

        for s in range(2, BPC):
            if s + 2 < BPC:
                xrs.append(load_x(s + 2))
            for dh in range(2):
                for br in range(3):
                    do_group(s, dh, br)

        stage2 = spool.tile([128, 3, 2, BPC], f32)
        nc.vector.tensor_tensor(
            stage2[:],
            stage[:],
            bt[:, :, :, None].to_broadcast((128, 3, 2, BPC)),
            mybir.AluOpType.add,
        )
        nc.vector.tensor_scalar_max(stage2[:], stage2[:], 0.0)
        nc.sync.dma_start(out_d[:], stage2[:])

    nc.compile()
    return nc


def _pack_inputs(input, W1, W2, W3, b1, b2, b3):
    # Host-materialized im2col: Xrep[b, k, t] = x[b, t + k//300, k%300],
    # laid out as 12 K-tiles of 128 rows, SEQ padded to 400 with zeros.
    xt = np.zeros((B, EMB, SEQP), np.float32)
    xt[:, :, :SEQ] = np.asarray(input, np.float32).transpose(0, 2, 1)
    xrep = np.zeros((B, KTOT * 128, SEQP), np.float32)
    for j in range(5):
        rows = xrep[:, j * EMB : (j + 1) * EMB, : SEQP - j]
        rows[:] = xt[:, :, j:]
    xt = xrep.reshape(B, KTOT, 128, SEQP).astype(np.float16)

    wq = np.zeros((128, 2, NCOL, 128), np.float32)  # cast to fp16 below
    for br, (n, W) in enumerate(zip(NS, (W1, W2, W3))):
        Wt = np.asarray(W, np.float32).T  # [n*300, 256]
        for r in range(NTILES[br]):
            rows = Wt[128 * r : min(128 * (r + 1), n * EMB)]
            for dh in range(2):
                wq[: rows.shape[0], dh, COLB[br] + r, :] = (
                    rows[:, dh * 128 : (dh + 1) * 128]
                )

    wq = wq.astype(np.float16)

    bp = np.empty((128, 3, 2), np.float32)
    for br, b in enumerate((b1, b2, b3)):
        b = np.asarray(b, np.float32).reshape(DEPTH)
        for dh in range(2):
            bp[:, br, dh] = b[dh * 128 : (dh + 1) * 128]
    return xt, wq, bp


def kernel(input, W1, W2, W3, b1, b2, b3):
    global _built, LAST_RESULT
    from concourse.bass_utils import run_bass_kernel_spmd

    xt, wq, bp = _pack_inputs(input, W1, W2, W3, b1, b2, b3)

    if _built is None:
        _built = _build_bass()
    nc = _built

    in_maps = [
        {"xt": xt[c * BPC : (c + 1) * BPC], "wq": wq, "bp": bp}
        for c in range(NCORES)
    ]
    res = run_bass_kernel_spmd(
        nc, in_maps, core_ids=list(range(NCORES)), trace=TRACE
    )
    LAST_RESULT = res

    out = np.empty((B, 3 * DEPTH), np.float32)
    for c in range(NCORES):
        arr = res.results[c]["out_t"]  # [128, 3, 2, BPC]
        out[c * BPC : (c + 1) * BPC] = arr.transpose(3, 1, 2, 0).reshape(BPC, 768)
    return out


# revision 27
# speedup vs baseline: 1.0176x; 1.0176x over previous
"""TextCNN-style conv layer (kernel sizes 3/4/5, EMB=300 -> DEPTH=256, bias,
ReLU, max-pool over time) as a Bass/Tile kernel for 8 Trainium2 NeuronCores.

Strategy: data-parallel over batch (8 samples per core), weights replicated.

Conv as dense-K matmuls: for branch n, window output
y[d, i] = sum_{k < n*300} Xrep[k, i] * Wn[d, k]  with  Xrep[k, i] =
x[i + k//300, k%300] -- the im2col matrix.  Xrep rows are materialized in
SBUF as 12 K-tiles of 128 rows per sample, built by <=2 shifted DMA segments
per tile straight from the transposed input in DRAM (a row (j, e) is just
x_t[e, j:] -- a free-dim offset), so no host-side replication and each
branch contracts over ceil(n*300/128) dense K=128 tiles (8/10/12 -> 30
matmuls per sample per depth-half vs 36 for the per-(j,chunk) split).
Branch boundaries that fall inside a tile are handled by zero-padding the
*weights* (the x rows there hold valid shifted data).  The final K-tile's
rows past 1500 are never written, so its matmuls contract only K=92.

dtype float32r: FP22 multiplies at full PE rate, fp32 PSUM accumulate; the
moving free-dim count must be even, so branches with odd SEQ-n compute one
extra (still valid) window that the max-reduce then ignores.

Epilogue: relu(max_i(y + b)) == max(0, max_i y + b): DVE reduce_max over the
window axis straight out of PSUM, broadcast bias add + clamp at 0, output
staged [d, branch, half, sample] per core and de-transposed on host.
"""

import numpy as np

B, SEQ, EMB = 64, 394, 300
DEPTH = 256
NCORES = 8
BPC = B // NCORES  # samples per core
SEQP = 400  # x_t free-dim padded (zeros) so shifted loads stay in bounds
NS = (3, 4, 5)
NTILES = (8, 10, 12)  # ceil(n*300/128) K-tiles per branch
COLB = (0, 8, 18)  # weight column base per branch
NCOL = 30
KTOT = 12  # distinct Xrep K-tiles per sample

# DMA segments building the 12 Xrep K-tiles: (tile r, p0, plen, j, e0)
_SEGS = []
for _r in range(KTOT):
    _k, _k1 = 128 * _r, min(128 * (_r + 1), 5 * EMB)
    while _k < _k1:
        _j, _e = divmod(_k, EMB)
        _plen = min(_k1 - _k, EMB - _e)
        _SEGS.append((_r, _k - 128 * _r, _plen, _j, _e))
        _k += _plen

TRACE = False
LAST_RESULT = None

_built = None


def _build_bass():
    import concourse.mybir as mybir
    import concourse.tile as tile
    from concourse import bacc
    from contextlib import ExitStack

    f32 = mybir.dt.float32
    f32r = mybir.dt.float32r
    f16 = mybir.dt.float16

    nc = bacc.Bacc("TRN2", target_bir_lowering=False)
    xt_d = nc.dram_tensor("xt", (BPC, KTOT, 128, SEQP), f16, kind="ExternalInput")
    wq_d = nc.dram_tensor("wq", (128, 2, NCOL, 128), f16, kind="ExternalInput")
    bp_d = nc.dram_tensor("bp", (128, 3, 2), f32, kind="ExternalInput")
    out_d = nc.dram_tensor("out_t", (128, 3, 2, BPC), f32, kind="ExternalOutput")

    with tile.TileContext(nc) as tc, ExitStack() as ctx:
        xpool = ctx.enter_context(tc.tile_pool(name="x", bufs=5))
        wpool = ctx.enter_context(tc.tile_pool(name="w", bufs=1))
        cpool = ctx.enter_context(tc.tile_pool(name="consts", bufs=1))
        spool = ctx.enter_context(tc.tile_pool(name="stage", bufs=1))
        pspool = ctx.enter_context(tc.tile_pool(name="ps", bufs=8, space="PSUM"))

        # x segments alternate over the two fast HWDGE rings (SP, ACT);
        # weights + bias go on the gpsimd SWDGE ring in parallel so they
        # never block the x pipeline.
        hw_engines = (nc.sync, nc.scalar)
        rr = [0]

        wts = {}

        def load_w(dh, br, eng):
            nt = NTILES[br]
            wt = wpool.tile([128, nt, 128], f16, tag=f"w{dh}{br}")
            eng.dma_start(wt[:], wq_d[:, dh, COLB[br] : COLB[br] + nt, :])
            wts[dh, br] = wt

        def load_x(s):
            # One pool tile + one contiguous DMA per K-tile: a matmul waits
            # only on the single DMA that wrote its contraction rows.
            xr = [
                xpool.tile([128, SEQP], f16, tag=f"x{r}", name=f"x{r}_{s}")
                for r in range(KTOT)
            ]
            for r in range(KTOT):
                eng = hw_engines[rr[0] % 2]
                rr[0] += 1
                eng.dma_start(xr[r][:], xt_d[s, r])
            return xr

        # The whole working set (3.9MB weights + 2.4MB im2col per sample)
        # drains from HBM at ~350GB/s, so the first ~25us are DMA-paced.
        # Interleave the loads in need-order and run samples 0-1 group-major
        # (stretching each weight tile's deadline) before switching to
        # sample-major for the pipelined steady state.
        load_w(0, 0, nc.sync)
        xrs = [load_x(0)]
        load_w(0, 1, nc.scalar)
        load_w(0, 2, nc.sync)
        load_w(1, 0, nc.scalar)
        load_w(1, 1, nc.sync)
        load_w(1, 2, nc.scalar)
        xrs.append(load_x(1))
        bt = cpool.tile([128, 3, 2], f32)
        nc.gpsimd.dma_start(bt[:], bp_d[:])
        xrs.append(load_x(2))

        stage = spool.tile([128, 3, 2, BPC], f32)

        def do_group(s, dh, br):
            n = NS[br]
            nw = SEQ - n  # windows the reference maxes over
            nmm = nw + (nw & 1)  # keep the moving count even
            nt = NTILES[br]
            xr = xrs[s]
            ps = pspool.tile([128, 512], f32, tag="ps", name=f"ps_{s}_{dh}_{br}")
            for r in range(nt):
                kk = min(128, 5 * EMB - 128 * r)  # 92 on the last tile
                nc.tensor.matmul(
                    ps[:, :nmm],
                    lhsT=wts[dh, br][:kk, r, :],
                    rhs=xr[r][:kk, :nmm],
                    start=(r == 0),
                    stop=(r == nt - 1),
                )
            nc.vector.reduce_max(
                stage[:, br, dh, s : s + 1],
                ps[:, :nw],
                axis=mybir.AxisListType.X,
            )

        for s in range(BPC):
            if 3 <= s + 3 < BPC + 3 and s + 3 < BPC:
                xrs.append(load_x(s + 3))
            for dh in range(2):
                for br in range(3):
                    do_group(s, dh, br)

        stage2 = spool.tile([128, 3, 2, BPC], f32)
        nc.vector.tensor_tensor(
            stage2[:],
            stage[:],
            bt[:, :, :, None].to_broadcast((128, 3, 2, BPC)),
            mybir.AluOpType.add,
        )
        nc.vector.tensor_scalar_max(stage2[:], stage2[:], 0.0)
        nc.sync.dma_start(out_d[:], stage2[:])

    nc.compile()
    return nc


def _pack_inputs(input, W1, W2, W3, b1, b2, b3):
    # Host-materialized im2col: Xrep[b, k, t] = x[b, t + k//300, k%300],
    # laid out as 12 K-tiles of 128 rows, SEQ padded to 400 with zeros.
    xt = np.zeros((B, EMB, SEQP), np.float32)
    xt[:, :, :SEQ] = np.asarray(input, np.float32).transpose(0, 2, 1)
    xrep = np.zeros((B, KTOT * 128, SEQP), np.float32)
    for j in range(5):
        rows = xrep[:, j * EMB : (j + 1) * EMB, : SEQP - j]
        rows[:] = xt[:, :, j:]
    xt = xrep.reshape(B, KTOT, 128, SEQP).astype(np.float16)

    wq = np.zeros((128, 2, NCOL, 128), np.float32)  # cast to fp16 below
    for br, (n, W) in enumerate(zip(NS, (W1, W2, W3))):
        Wt = np.asarray(W, np.float32).T  # [n*300, 256]
        for r in range(NTILES[br]):
            rows = Wt[128 * r : min(128 * (r + 1), n * EMB)]
            for dh in range(2):
                wq[: rows.shape[0], dh, COLB[br] + r, :] = (
                    rows[:, dh * 128 : (dh + 1) * 128]
                )

    wq = wq.astype(np.float16)

    bp = np.empty((128, 3, 2), np.float32)
    for br, b in enumerate((b1, b2, b3)):
        b = np.asarray(b, np.float32).reshape(DEPTH)
        for dh in range(2):
            bp[:, br, dh] = b[dh * 128 : (dh + 1) * 128]
    return xt, wq, bp


def kernel(input, W1, W2, W3, b1, b2, b3):
    global _built, LAST_RESULT
    from concourse.bass_utils import run_bass_kernel_spmd

    xt, wq, bp = _pack_inputs(input, W1, W2, W3, b1, b2, b3)

    if _built is None:
        _built = _build_bass()
    nc = _built

    in_maps = [
        {"xt": xt[c * BPC : (c + 1) * BPC], "wq": wq, "bp": bp}
        for c in range(NCORES)
    ]
    res = run_bass_kernel_spmd(
        nc, in_maps, core_ids=list(range(NCORES)), trace=TRACE
    )
    LAST_RESULT = res

    out = np.empty((B, 3 * DEPTH), np.float32)
    for c in range(NCORES):
        arr = res.results[c]["out_t"]  # [128, 3, 2, BPC]
        out[c * BPC : (c + 1) * BPC] = arr.transpose(3, 1, 2, 0).reshape(BPC, 768)
    return out
